# revision 7
# baseline (speedup 1.0000x reference)
"""Trainium2 Bass kernel for nn_Divergence2d.

Math (from the reference):
  q = C//4 = 4 channel groups A=x[:, :4], B=x[:,4:8], C=x[:,8:12], D=x[:,12:16]
  With per-group channel sums  Asum(r,c) = sum_ch lam_ch x[ch, r, c]  (lam only
  for group A):

    out1[i,j] = lam*(Asum[i-1, j] - Asum[i-1, j-2]) + Bsum[i-2, j-1] - Bsum[i, j-1]
    out2[i,j] =     (Csum[i-1, j] - Csum[i-1, j-2]) + Dsum[i-2, j-1] - Dsum[i, j-1]

  for i,j in [0, 514), with zero padding outside [0,512).

Strategy (v3, pure data parallel, 2 images per core on 8 cores):
  The op is memory-bound (38 MB HBM traffic/core; measured free-running DMA
  rate is ~335 GB/s with this access pattern -> ~114 us floor).  The previous
  kernel did all stencil work with fp32 matmuls (1/4 PE rate) which kept
  TensorE 76% busy and paced the DMA down to ~205 GB/s effective.  This
  version makes every compute engine cheap so the DMA free-runs:

  - per 126-row block, ONE 4 MB HWDGE load of a 128-row window of all 16
    channels into [128 rows, 16ch x 512] (descriptor order [row, ch] ->
    each of the 16 DMA engines streams one channel sequentially from HBM)
  - DVE (fp32): channel sums via 2+2 grouped adds (3D APs), then the
    *horizontal* stencil diff as free-dim-shifted ops, writing bf16 maps
    (hA = Asum[:, j] - Asum[:, j-2]; hB = Bsum[:, j-1] via padded layout)
  - TensorE (bf16, full rate): the *vertical* shifts as one-hot shift
    matmuls (weights in {0,+-1}, exact in bf16): psA[m] = hA[m+1],
    psB[m] = hB[m] - hB[m+2].  Compute-engine APs cannot start at a
    nonzero partition on TRN2, so row shifts must go through the PE.
  - ACT drains the B/D psums to SBUF; DVE combines (lam*psA + sB) into the
    output tile; one HWDGE store per block.

  Only the maps are bf16-rounded (weights are exact), giving ~4e-3 l2 rel
  error vs the 2e-2 gate.  All engines run at <40% of the DMA time.
"""
import sys

for _p in (
    "/root/.axon_site",
    "/root/.axon_site/_ro/trn_rl_repo",
    "/root/.axon_site/_ro/pypackages",
    "/opt/trn_rl_repo",
):
    if _p not in sys.path:
        sys.path.append(_p)

import numpy as np

N_CORES = 8
N, C, H, W = 16, 16, 512, 512
PB = N // N_CORES          # images per core
HO = WO = H + 2            # 514
BLK = 126                  # output rows per block
BLOCKS = []
_i0 = 0
while _i0 < HO:
    BLOCKS.append((_i0, min(BLK, HO - _i0)))
    _i0 += BLK
# -> [(0,126), (126,126), (252,126), (378,126), (504,10)]

_cache = {}


def _build(lam4):
    import concourse.bacc as bacc
    import concourse.mybir as mybir
    from concourse.tile import TileContext

    f32 = mybir.dt.float32
    bf16 = mybir.dt.bfloat16
    ALU = mybir.AluOpType
    ACT_COPY = mybir.ActivationFunctionType.Copy
    lam_eq = all(float(v) == float(lam4[0]) for v in lam4)
    lam0 = float(lam4[0])

    nc = bacc.Bacc("TRN2", target_bir_lowering=False, debug=False,
                   num_devices=N_CORES, detect_race_conditions=False)
    x = nc.dram_tensor("x", (PB, C, H, W), f32, kind="ExternalInput")
    out = nc.dram_tensor("out", (PB, 2, HO, WO), f32, kind="ExternalOutput")

    with TileContext(nc) as tc:
        with (
            tc.tile_pool(name="consts", bufs=1) as c_pool,
            tc.tile_pool(name="rhs", bufs=4) as rhs_pool,
            tc.tile_pool(name="work", bufs=1) as w_pool,
            tc.tile_pool(name="psum", bufs=1, space="PSUM") as ps_pool,
            tc.tile_pool(name="outs", bufs=3) as out_pool,
        ):
            # ---- one-time shift weights [128 rows, 126 out rows], bf16 ----
            R = c_pool.tile([128, BLK], f32, tag="R")
            nc.gpsimd.iota(R[:, :], pattern=[[0, BLK]], base=0,
                           channel_multiplier=1,
                           allow_small_or_imprecise_dtypes=True)
            Sm = []
            for b in range(3):                               # m+0, m+1, m+2
                t_ = c_pool.tile([128, BLK], f32, tag=f"Sm{b}", name=f"Sm{b}")
                nc.gpsimd.iota(t_[:, :], pattern=[[1, BLK]], base=b,
                               channel_multiplier=0,
                               allow_small_or_imprecise_dtypes=True)
                Sm.append(t_)
            S1 = c_pool.tile([128, BLK], bf16, tag="S1")     # d(r, m+1)
            nc.vector.tensor_tensor(S1[:, :], R[:, :], Sm[1][:, :], ALU.is_equal)
            e0 = c_pool.tile([128, BLK], f32, tag="e0")
            e2 = c_pool.tile([128, BLK], f32, tag="e2")
            nc.vector.tensor_tensor(e0[:, :], R[:, :], Sm[0][:, :], ALU.is_equal)
            nc.vector.tensor_tensor(e2[:, :], R[:, :], Sm[2][:, :], ALU.is_equal)
            Sbd = c_pool.tile([128, BLK], bf16, tag="Sbd")   # d(r,m)-d(r,m+2)
            nc.vector.tensor_tensor(Sbd[:, :], e0[:, :], e2[:, :], ALU.subtract)

            # ---- persistent work tiles (single-buffered; DVE is in-order) --
            s1 = w_pool.tile([128, 4 * 1024], f32, tag="s1")
            mpAC = w_pool.tile([128, 2 * 516], f32, tag="mpAC")
            hAC = w_pool.tile([128, 2 * WO], bf16, tag="hAC")
            hBD = w_pool.tile([128, 2 * WO], bf16, tag="hBD")
            sBD = w_pool.tile([128, 2 * 512], f32, tag="sBD")
            sE = w_pool.tile([128, 4], f32, tag="sE")
            if not lam_eq:
                tA = w_pool.tile([128, 4 * 512], f32, tag="tA")
            s1v = s1[:, :].rearrange("p (g k) -> p g k", k=1024)
            mpv = mpAC[:, :].rearrange("p (m c) -> p m c", c=516)
            hBDv = hBD[:, :].rearrange("p (m c) -> p m c", c=WO)

            # zero the pads once; data ops below never write them.
            # mpAC (A,C fp32): data at cols [2,514) -> pads {0,1,514,515}
            # hBD  (B,D bf16): data at cols [1,513) -> pads {0,513}
            nc.vector.memset(mpv[:, 0:2, 0:2], 0.0)
            nc.vector.memset(mpv[:, 0:2, 514:516], 0.0)
            nc.vector.memset(hBDv[:, 0:2, 0:1], 0.0)
            nc.vector.memset(hBDv[:, 0:2, 513:514], 0.0)

            # ---- main loop ---------------------------------------------
            for n in range(PB):
                for i0, nr in BLOCKS:
                    r0 = i0 - 2                 # window row r <-> x row r0+r
                    rlo, rhi = max(r0, 0), min(r0 + 128, H)
                    p0, npart = rlo - r0, rhi - rlo
                    np_use = min(nr + 2, 128)   # window rows the maps read
                    tail = p0 + npart < np_use
                    t = rhs_pool.tile([128, 16 * 512], f32, tag="rhs")
                    if p0 > 0:
                        nc.vector.memset(t[0:p0, :], 0.0)
                    if tail:
                        # zero-pad rows live below the valid ones; APs must
                        # start at partition 0, so zero the (small) map tiles
                        # and keep the sums to the valid rows instead of
                        # memsetting the 32 KB/partition window tile
                        nc.vector.memset(mpv[0:np_use, :, :], 0.0)
                        nc.vector.memset(hBDv[0:np_use, :, :], 0.0)
                    tv = t[:, :].rearrange("p (c w) -> p c w", w=512)
                    nc.sync.dma_start(out=tv[p0:p0 + npart, :, :],
                                      in_=x[n, :, rlo:rhi, :].rearrange(
                                          "c r w -> r c w"))

                    Ps = p0 + npart if tail else np_use   # rows with data
                    P = np_use                            # rows the PE reads
                    tg = t[:, :].rearrange("p (g k) -> p g k", k=2048)
                    # -- channel sums: A,C on DVE; B,D on GpSimd (parallel) --
                    if lam_eq:
                        nc.vector.tensor_tensor(
                            s1v[0:Ps, 0:3:2, :], tg[0:Ps, 0:3:2, 0:1024],
                            tg[0:Ps, 0:3:2, 1024:2048], ALU.add)
                    else:
                        tAv = tA[:, :].rearrange("p (c w) -> p c w", w=512)
                        for c4 in range(4):
                            nc.vector.tensor_scalar_mul(
                                tAv[0:Ps, c4, :], tv[0:Ps, c4, :], float(lam4[c4]))
                        nc.vector.tensor_tensor(
                            s1[0:Ps, 0:1024], tA[0:Ps, 0:1024],
                            tA[0:Ps, 1024:2048], ALU.add)
                        nc.vector.tensor_tensor(
                            s1v[0:Ps, 2:3, :], tg[0:Ps, 2:3, 0:1024],
                            tg[0:Ps, 2:3, 1024:2048], ALU.add)
                    nc.gpsimd.tensor_tensor(
                        s1v[0:Ps, 1:4:2, :], tg[0:Ps, 1:4:2, 0:1024],
                        tg[0:Ps, 1:4:2, 1024:2048], ALU.add)
                    # A,C sums into padded fp32 maps (s1 groups 0,2)
                    nc.vector.tensor_tensor(
                        mpv[0:Ps, 0:2, 2:514], s1v[0:Ps, 0:3:2, 0:512],
                        s1v[0:Ps, 0:3:2, 512:1024], ALU.add)
                    # horizontal diff -> bf16: hA[p,j] = Asum[p,j]-Asum[p,j-2]
                    hACv = hAC[:, :].rearrange("p (m c) -> p m c", c=WO)
                    nc.vector.tensor_tensor(
                        hACv[0:P, 0:2, :], mpv[0:P, 0:2, 2:516],
                        mpv[0:P, 0:2, 0:514], ALU.subtract)
                    # B,D sums straight into padded bf16 maps (s1 groups 1,3)
                    nc.gpsimd.tensor_tensor(
                        hBDv[0:Ps, 0:2, 1:513], s1v[0:Ps, 1:4:2, 0:512],
                        s1v[0:Ps, 1:4:2, 512:1024], ALU.add)

                    # -- vertical shifts on the PE (bf16, one-hot weights) --
                    psA = ps_pool.tile([128, 512], f32, tag="psA", name="psA")
                    psC = ps_pool.tile([128, 512], f32, tag="psC", name="psC")
                    psB = ps_pool.tile([128, 512], f32, tag="psB", name="psB")
                    psD = ps_pool.tile([128, 512], f32, tag="psD", name="psD")
                    psE = ps_pool.tile([128, 8], f32, tag="psE", name="psE")
                    # S1 group: psA[m]=hA[m+1], interior cols then 2 edge cols
                    nc.tensor.matmul(psA[0:nr, :], S1[:, 0:nr],
                                     hAC[:, 0:512], start=True, stop=True)
                    nc.tensor.matmul(psC[0:nr, :], S1[:, 0:nr],
                                     hAC[:, WO:WO + 512], start=True, stop=True)
                    nc.tensor.matmul(psE[0:nr, 0:2], S1[:, 0:nr],
                                     hAC[:, 512:514], start=True, stop=True)
                    nc.tensor.matmul(psE[0:nr, 2:4], S1[:, 0:nr],
                                     hAC[:, WO + 512:WO + 514],
                                     start=True, stop=True)
                    # Sbd group: psB[m]=hB[m]-hB[m+2]
                    nc.tensor.matmul(psB[0:nr, :], Sbd[:, 0:nr],
                                     hBD[:, 0:512], start=True, stop=True)
                    nc.tensor.matmul(psD[0:nr, :], Sbd[:, 0:nr],
                                     hBD[:, WO:WO + 512], start=True, stop=True)
                    nc.tensor.matmul(psE[0:nr, 4:6], Sbd[:, 0:nr],
                                     hBD[:, 512:514], start=True, stop=True)
                    nc.tensor.matmul(psE[0:nr, 6:8], Sbd[:, 0:nr],
                                     hBD[:, WO + 512:WO + 514],
                                     start=True, stop=True)

                    # -- drain B/D (ACT), combine (DVE), store --
                    nc.scalar.activation(sBD[0:nr, 0:512], psB[0:nr, :],
                                         ACT_COPY)
                    nc.scalar.activation(sBD[0:nr, 512:1024], psD[0:nr, :],
                                         ACT_COPY)
                    nc.scalar.activation(sE[0:nr, :], psE[0:nr, 4:8], ACT_COPY)
                    o = out_pool.tile([128, 2 * WO], f32, tag="o")
                    if lam_eq:
                        nc.vector.scalar_tensor_tensor(
                            o[0:nr, 0:512], psA[0:nr, :], lam0,
                            sBD[0:nr, 0:512], ALU.mult, ALU.add)
                        nc.vector.scalar_tensor_tensor(
                            o[0:nr, 512:514], psE[0:nr, 0:2], lam0,
                            sE[0:nr, 0:2], ALU.mult, ALU.add)
                    else:
                        nc.vector.tensor_tensor(
                            o[0:nr, 0:512], psA[0:nr, :], sBD[0:nr, 0:512],
                            ALU.add)
                        nc.vector.tensor_tensor(
                            o[0:nr, 512:514], psE[0:nr, 0:2], sE[0:nr, 0:2],
                            ALU.add)
                    nc.vector.tensor_tensor(
                        o[0:nr, WO:WO + 512], psC[0:nr, :], sBD[0:nr, 512:1024],
                        ALU.add)
                    nc.vector.tensor_tensor(
                        o[0:nr, WO + 512:2 * WO], psE[0:nr, 2:4], sE[0:nr, 2:4],
                        ALU.add)
                    osrc = o[0:nr, :].rearrange("p (ch w) -> p ch w", w=WO)
                    ov = out[n].rearrange("ch r w -> r ch w")
                    nc.scalar.dma_start(out=ov[i0:i0 + nr, :, :], in_=osrc)
    nc.finalize()
    return nc


def _get_nc(lam4):
    key = tuple(float(v) for v in lam4)
    if key not in _cache:
        _cache[key] = _build(key)
    return _cache[key]


def _run(xs: np.ndarray, lam4, trace: bool = False, tmpdir=None):
    from concourse.bass_utils import run_bass_kernel_spmd

    nc = _get_nc(lam4)
    in_maps = [{"x": np.ascontiguousarray(xs[PB * c:PB * (c + 1)])}
               for c in range(N_CORES)]
    res = run_bass_kernel_spmd(nc, in_maps, list(range(N_CORES)),
                               trace=trace, tmpdir=tmpdir)
    full = np.concatenate([res.results[c]["out"] for c in range(N_CORES)], axis=0)
    return full, res


def kernel(x, lam1x, lam2x, lam1y, lam2y):
    x = np.ascontiguousarray(np.asarray(x, dtype=np.float32))
    assert x.shape == (N, C, H, W), x.shape
    lam4 = np.asarray(lam1x, dtype=np.float32).reshape(-1)
    assert lam4.shape == (4,), lam4.shape
    full, _ = _run(x, lam4)
    return full


# revision 8
# speedup vs baseline: 1.2651x; 1.2651x over previous
"""Trainium2 Bass kernel for nn_Divergence2d.

Math (from the reference):
  q = C//4 = 4 channel groups A=x[:, :4], B=x[:,4:8], C=x[:,8:12], D=x[:,12:16]
  With per-group channel sums  Asum(r,c) = sum_ch lam_ch x[ch, r, c]  (lam only
  for group A):

    out1[i,j] = lam*(Asum[i-1, j] - Asum[i-1, j-2]) + Bsum[i-2, j-1] - Bsum[i, j-1]
    out2[i,j] =     (Csum[i-1, j] - Csum[i-1, j-2]) + Dsum[i-2, j-1] - Dsum[i, j-1]

  for i,j in [0, 514), with zero padding outside [0,512).

Strategy (v3, pure data parallel, 2 images per core on 8 cores):
  The op is memory-bound (38 MB HBM traffic/core; measured free-running DMA
  rate is ~335 GB/s with this access pattern -> ~114 us floor).  The previous
  kernel did all stencil work with fp32 matmuls (1/4 PE rate) which kept
  TensorE 76% busy and paced the DMA down to ~205 GB/s effective.  This
  version makes every compute engine cheap so the DMA free-runs:

  - per 126-row block, ONE 4 MB HWDGE load of a 128-row window of all 16
    channels into [128 rows, 16ch x 512] (descriptor order [row, ch] ->
    each of the 16 DMA engines streams one channel sequentially from HBM)
  - DVE (fp32): channel sums via 2+2 grouped adds (3D APs), then the
    *horizontal* stencil diff as free-dim-shifted ops, writing bf16 maps
    (hA = Asum[:, j] - Asum[:, j-2]; hB = Bsum[:, j-1] via padded layout)
  - TensorE (bf16, full rate): the *vertical* shifts as one-hot shift
    matmuls (weights in {0,+-1}, exact in bf16): psA[m] = hA[m+1],
    psB[m] = hB[m] - hB[m+2].  Compute-engine APs cannot start at a
    nonzero partition on TRN2, so row shifts must go through the PE.
  - ACT drains the B/D psums to SBUF; DVE combines (lam*psA + sB) into the
    output tile; one HWDGE store per block.

  Only the maps are bf16-rounded (weights are exact), giving ~4e-3 l2 rel
  error vs the 2e-2 gate.  All engines run at <40% of the DMA time.
"""
import sys

for _p in (
    "/root/.axon_site",
    "/root/.axon_site/_ro/trn_rl_repo",
    "/root/.axon_site/_ro/pypackages",
    "/opt/trn_rl_repo",
):
    if _p not in sys.path:
        sys.path.append(_p)

import numpy as np

N_CORES = 8
N, C, H, W = 16, 16, 512, 512
PB = N // N_CORES          # images per core
HO = WO = H + 2            # 514
BLK = 126                  # output rows per block
BLOCKS = []
_i0 = 0
while _i0 < HO:
    BLOCKS.append((_i0, min(BLK, HO - _i0)))
    _i0 += BLK
# -> [(0,126), (126,126), (252,126), (378,126), (504,10)]

_cache = {}


def _build(lam4):
    import concourse.bacc as bacc
    import concourse.mybir as mybir
    from concourse.tile import TileContext

    f32 = mybir.dt.float32
    bf16 = mybir.dt.bfloat16
    ALU = mybir.AluOpType
    ACT_COPY = mybir.ActivationFunctionType.Copy
    lam_eq = all(float(v) == float(lam4[0]) for v in lam4)
    lam0 = float(lam4[0])

    nc = bacc.Bacc("TRN2", target_bir_lowering=False, debug=False,
                   num_devices=N_CORES, detect_race_conditions=False)
    x = nc.dram_tensor("x", (PB, C, H, W), f32, kind="ExternalInput")
    out = nc.dram_tensor("out", (PB, 2, HO, WO), f32, kind="ExternalOutput")

    with TileContext(nc) as tc:
        with (
            tc.tile_pool(name="consts", bufs=1) as c_pool,
            tc.tile_pool(name="rhs", bufs=4) as rhs_pool,
            tc.tile_pool(name="work", bufs=1) as w_pool,
            tc.tile_pool(name="psum", bufs=1, space="PSUM") as ps_pool,
            tc.tile_pool(name="outs", bufs=3) as out_pool,
        ):
            # ---- one-time shift weights [128 rows, 126 out rows], bf16 ----
            R = c_pool.tile([128, BLK], f32, tag="R")
            nc.gpsimd.iota(R[:, :], pattern=[[0, BLK]], base=0,
                           channel_multiplier=1,
                           allow_small_or_imprecise_dtypes=True)
            Sm = []
            for b in range(3):                               # m+0, m+1, m+2
                t_ = c_pool.tile([128, BLK], f32, tag=f"Sm{b}", name=f"Sm{b}")
                nc.gpsimd.iota(t_[:, :], pattern=[[1, BLK]], base=b,
                               channel_multiplier=0,
                               allow_small_or_imprecise_dtypes=True)
                Sm.append(t_)
            S1 = c_pool.tile([128, BLK], bf16, tag="S1")     # d(r, m+1)
            nc.vector.tensor_tensor(S1[:, :], R[:, :], Sm[1][:, :], ALU.is_equal)
            e0 = c_pool.tile([128, BLK], f32, tag="e0")
            e2 = c_pool.tile([128, BLK], f32, tag="e2")
            nc.vector.tensor_tensor(e0[:, :], R[:, :], Sm[0][:, :], ALU.is_equal)
            nc.vector.tensor_tensor(e2[:, :], R[:, :], Sm[2][:, :], ALU.is_equal)
            Sbd = c_pool.tile([128, BLK], bf16, tag="Sbd")   # d(r,m)-d(r,m+2)
            nc.vector.tensor_tensor(Sbd[:, :], e0[:, :], e2[:, :], ALU.subtract)

            # ---- persistent work tiles (single-buffered; DVE is in-order) --
            s1 = w_pool.tile([128, 4 * 1024], f32, tag="s1")
            mpAC = w_pool.tile([128, 2 * 516], f32, tag="mpAC")
            hAC = w_pool.tile([128, 2 * WO], bf16, tag="hAC")
            hBD = w_pool.tile([128, 2 * WO], bf16, tag="hBD")
            sBD = w_pool.tile([128, 2 * 512], f32, tag="sBD")
            sE = w_pool.tile([128, 4], f32, tag="sE")
            if not lam_eq:
                tA = w_pool.tile([128, 4 * 512], f32, tag="tA")
            s1v = s1[:, :].rearrange("p (g k) -> p g k", k=1024)
            mpv = mpAC[:, :].rearrange("p (m c) -> p m c", c=516)
            hBDv = hBD[:, :].rearrange("p (m c) -> p m c", c=WO)

            # zero the pads once; data ops below never write them.
            # mpAC (A,C fp32): data at cols [2,514) -> pads {0,1,514,515}
            # hBD  (B,D bf16): data at cols [1,513) -> pads {0,513}
            nc.vector.memset(mpv[:, 0:2, 0:2], 0.0)
            nc.vector.memset(mpv[:, 0:2, 514:516], 0.0)
            nc.vector.memset(hBDv[:, 0:2, 0:1], 0.0)
            nc.vector.memset(hBDv[:, 0:2, 513:514], 0.0)

            # ---- main loop ---------------------------------------------
            for n in range(PB):
                for i0, nr in BLOCKS:
                    r0 = i0 - 2                 # window row r <-> x row r0+r
                    rlo, rhi = max(r0, 0), min(r0 + 128, H)
                    p0, npart = rlo - r0, rhi - rlo
                    np_use = min(nr + 2, 128)   # window rows the maps read
                    tail = p0 + npart < np_use
                    t = rhs_pool.tile([128, 16 * 512], f32, tag="rhs")
                    if p0 > 0:
                        nc.vector.memset(t[0:p0, :], 0.0)
                    if tail:
                        # zero-pad rows live below the valid ones; APs must
                        # start at partition 0, so zero the (small) map tiles
                        # and keep the sums to the valid rows instead of
                        # memsetting the 32 KB/partition window tile
                        nc.vector.memset(mpv[0:np_use, :, :], 0.0)
                        nc.vector.memset(hBDv[0:np_use, :, :], 0.0)
                    tv = t[:, :].rearrange("p (c w) -> p c w", w=512)
                    nc.sync.dma_start(out=tv[p0:p0 + npart, :, :],
                                      in_=x[n, :, rlo:rhi, :].rearrange(
                                          "c r w -> r c w"))

                    Ps = p0 + npart if tail else np_use   # rows with data
                    P = np_use                            # rows the PE reads
                    tg = t[:, :].rearrange("p (g k) -> p g k", k=2048)
                    # -- channel sums (fp32, all DVE: other engines contend
                    #    for SBUF ports and slow everything down) --
                    if lam_eq:
                        nc.vector.tensor_tensor(
                            s1[0:Ps, :], tg[0:Ps, 0:4, 0:1024],
                            tg[0:Ps, 0:4, 1024:2048], ALU.add)
                    else:
                        tAv = tA[:, :].rearrange("p (c w) -> p c w", w=512)
                        for c4 in range(4):
                            nc.vector.tensor_scalar_mul(
                                tAv[0:Ps, c4, :], tv[0:Ps, c4, :], float(lam4[c4]))
                        nc.vector.tensor_tensor(
                            s1[0:Ps, 0:1024], tA[0:Ps, 0:1024],
                            tA[0:Ps, 1024:2048], ALU.add)
                        nc.vector.tensor_tensor(
                            s1v[0:Ps, 1:4, :], tg[0:Ps, 1:4, 0:1024],
                            tg[0:Ps, 1:4, 1024:2048], ALU.add)
                    # A,C sums into padded fp32 maps (s1 groups 0,2)
                    nc.vector.tensor_tensor(
                        mpv[0:Ps, 0:2, 2:514], s1v[0:Ps, 0:3:2, 0:512],
                        s1v[0:Ps, 0:3:2, 512:1024], ALU.add)
                    # horizontal diff -> bf16: hA[p,j] = Asum[p,j]-Asum[p,j-2]
                    hACv = hAC[:, :].rearrange("p (m c) -> p m c", c=WO)
                    nc.vector.tensor_tensor(
                        hACv[0:P, 0:2, :], mpv[0:P, 0:2, 2:516],
                        mpv[0:P, 0:2, 0:514], ALU.subtract)
                    # B,D sums straight into padded bf16 maps (s1 groups 1,3)
                    nc.vector.tensor_tensor(
                        hBDv[0:Ps, 0:2, 1:513], s1v[0:Ps, 1:4:2, 0:512],
                        s1v[0:Ps, 1:4:2, 512:1024], ALU.add)

                    # -- vertical shifts on the PE (bf16, one-hot weights) --
                    psA = ps_pool.tile([128, 512], f32, tag="psA", name="psA")
                    psC = ps_pool.tile([128, 512], f32, tag="psC", name="psC")
                    psB = ps_pool.tile([128, 512], f32, tag="psB", name="psB")
                    psD = ps_pool.tile([128, 512], f32, tag="psD", name="psD")
                    psE = ps_pool.tile([128, 8], f32, tag="psE", name="psE")
                    # S1 group: psA[m]=hA[m+1], interior cols then 2 edge cols
                    nc.tensor.matmul(psA[0:nr, :], S1[:, 0:nr],
                                     hAC[:, 0:512], start=True, stop=True)
                    nc.tensor.matmul(psC[0:nr, :], S1[:, 0:nr],
                                     hAC[:, WO:WO + 512], start=True, stop=True)
                    nc.tensor.matmul(psE[0:nr, 0:2], S1[:, 0:nr],
                                     hAC[:, 512:514], start=True, stop=True)
                    nc.tensor.matmul(psE[0:nr, 2:4], S1[:, 0:nr],
                                     hAC[:, WO + 512:WO + 514],
                                     start=True, stop=True)
                    # Sbd group: psB[m]=hB[m]-hB[m+2]
                    nc.tensor.matmul(psB[0:nr, :], Sbd[:, 0:nr],
                                     hBD[:, 0:512], start=True, stop=True)
                    nc.tensor.matmul(psD[0:nr, :], Sbd[:, 0:nr],
                                     hBD[:, WO:WO + 512], start=True, stop=True)
                    nc.tensor.matmul(psE[0:nr, 4:6], Sbd[:, 0:nr],
                                     hBD[:, 512:514], start=True, stop=True)
                    nc.tensor.matmul(psE[0:nr, 6:8], Sbd[:, 0:nr],
                                     hBD[:, WO + 512:WO + 514],
                                     start=True, stop=True)

                    # -- drain B/D (ACT), combine (DVE), store --
                    nc.scalar.activation(sBD[0:nr, 0:512], psB[0:nr, :],
                                         ACT_COPY)
                    nc.scalar.activation(sBD[0:nr, 512:1024], psD[0:nr, :],
                                         ACT_COPY)
                    nc.scalar.activation(sE[0:nr, :], psE[0:nr, 4:8], ACT_COPY)
                    o = out_pool.tile([128, 2 * WO], f32, tag="o")
                    if lam_eq:
                        nc.vector.scalar_tensor_tensor(
                            o[0:nr, 0:512], psA[0:nr, :], lam0,
                            sBD[0:nr, 0:512], ALU.mult, ALU.add)
                        nc.vector.scalar_tensor_tensor(
                            o[0:nr, 512:514], psE[0:nr, 0:2], lam0,
                            sE[0:nr, 0:2], ALU.mult, ALU.add)
                    else:
                        nc.vector.tensor_tensor(
                            o[0:nr, 0:512], psA[0:nr, :], sBD[0:nr, 0:512],
                            ALU.add)
                        nc.vector.tensor_tensor(
                            o[0:nr, 512:514], psE[0:nr, 0:2], sE[0:nr, 0:2],
                            ALU.add)
                    nc.vector.tensor_tensor(
                        o[0:nr, WO:WO + 512], psC[0:nr, :], sBD[0:nr, 512:1024],
                        ALU.add)
                    nc.vector.tensor_tensor(
                        o[0:nr, WO + 512:2 * WO], psE[0:nr, 2:4], sE[0:nr, 2:4],
                        ALU.add)
                    osrc = o[0:nr, :].rearrange("p (ch w) -> p ch w", w=WO)
                    ov = out[n].rearrange("ch r w -> r ch w")
                    nc.scalar.dma_start(out=ov[i0:i0 + nr, :, :], in_=osrc)
    nc.finalize()
    return nc


def _get_nc(lam4):
    key = tuple(float(v) for v in lam4)
    if key not in _cache:
        _cache[key] = _build(key)
    return _cache[key]


def _run(xs: np.ndarray, lam4, trace: bool = False, tmpdir=None):
    from concourse.bass_utils import run_bass_kernel_spmd

    nc = _get_nc(lam4)
    in_maps = [{"x": np.ascontiguousarray(xs[PB * c:PB * (c + 1)])}
               for c in range(N_CORES)]
    res = run_bass_kernel_spmd(nc, in_maps, list(range(N_CORES)),
                               trace=trace, tmpdir=tmpdir)
    full = np.concatenate([res.results[c]["out"] for c in range(N_CORES)], axis=0)
    return full, res


def kernel(x, lam1x, lam2x, lam1y, lam2y):
    x = np.ascontiguousarray(np.asarray(x, dtype=np.float32))
    assert x.shape == (N, C, H, W), x.shape
    lam4 = np.asarray(lam1x, dtype=np.float32).reshape(-1)
    assert lam4.shape == (4,), lam4.shape
    full, _ = _run(x, lam4)
    return full
